# revision 5
# baseline (speedup 1.0000x reference)
"""Trainium2 Bass kernel for nn_LocallyDense (grouped gather + per-group Dense
+ LeakyReLU + BatchNorm inference).

Sharding: expert-parallel over the 41 groups across 8 cores (6 groups/core,
padded with a duplicate group on 5-group cores so one SPMD program fits all).

The gather runs on the HOST during sharding prep: each core receives its
groups' x-columns already gathered AND packed into tile layout
([128, NG*KT*B]: partition = k%128, free = (k-block, batch)), so the device
program is a pure streaming grouped GEMM with a TRANSPOSED output
(psum[o, b] via lhsT=W): per-output-channel constants (bias, BN scale/shift)
are then per-partition scalars riding the ACT/DVE instructions — no
broadcasts, no bias matmuls.

Per output half (z = x@W + b, p = psum = x@W):
  rt = Relu((1-a)*p + (1-a)*b)        # ACT, per-partition bias AP
  ot = a*p + rt = leaky(z) - a*b      # DVE scalar_tensor_tensor
  y  = ot*inv + (c + a*b*inv)         # DVE tensor_scalar, per-partition APs
where inv = gamma/sqrt(var+eps), c = beta - mean*inv (host-computed).
"""

import numpy as np
import ml_dtypes

B, D_IN, N_GROUPS, G, D_OUT = 256, 65536, 41, 1536, 256
BN_EPS = 1e-3
ALPHA = 0.3
N_CORES = 8
NG = 6                # groups per core (padded)
KT = G // 128         # 12 K-tiles per group
NCOL = NG * 2 * 2 + 2  # const table columns: biasS | c2 | inv

USE_BF16 = True       # x/W feed the PE in bf16 (fp32 accumulate in PSUM)
TRACE = False         # set by test.py for profiling runs
TRACE_KW = {}
REPEAT = 1

_prog_cache = {}


def _np_dtx():
    return ml_dtypes.bfloat16 if USE_BF16 else np.float32


def _build_program(use_bf16: bool):
    import concourse.bacc as bacc
    import concourse.mybir as mybir
    import concourse.tile as tile

    f32 = mybir.dt.float32
    dt_x = mybir.dt.bfloat16 if use_bf16 else mybir.dt.float32

    nc = bacc.Bacc("TRN2", target_bir_lowering=False, debug=False,
                   num_devices=N_CORES)
    xg = nc.dram_tensor("xg", [128, NG * KT * B], dt_x, kind="ExternalInput")
    wt = nc.dram_tensor("wt", [128, NG * KT * D_OUT], dt_x,
                        kind="ExternalInput")
    # columns: [0:12] biasS=(1-a)*b, [12:24] c2=c+a*b*inv, [24:26] inv
    cst = nc.dram_tensor("cst", [128, NCOL], f32, kind="ExternalInput")
    out = nc.dram_tensor("out", [NG * 2 * 128, B], f32, kind="ExternalOutput")

    with tile.TileContext(nc) as tc:
        with tc.tile_pool(name="const", bufs=1) as cpool, \
             tc.tile_pool(name="xp", bufs=2) as xpool, \
             tc.tile_pool(name="wp", bufs=2) as wpool, \
             tc.tile_pool(name="ep", bufs=4) as epool, \
             tc.tile_pool(name="ps", bufs=3, space="PSUM") as ppool:

            cst_t = cpool.tile([128, NCOL], f32)
            nc.sync.dma_start(out=cst_t[:], in_=cst[:, :])

            for g_rep in range(REPEAT * NG):
                g = g_rep % NG
                xt_g = xpool.tile([128, KT * B], dt_x, tag="xg")
                nc.sync.dma_start(
                    out=xt_g[:], in_=xg[:, g * KT * B:(g + 1) * KT * B])
                wt_g = wpool.tile([128, KT * D_OUT], dt_x, tag="wg")
                nc.sync.dma_start(
                    out=wt_g[:], in_=wt[:, g * KT * D_OUT:(g + 1) * KT * D_OUT])
                for h in range(2):
                    gh = g * 2 + h
                    ps = ppool.tile([128, B], f32, tag=f"ps{h}",
                                    name=f"ps{h}_{g_rep}")
                    for blk in range(KT):
                        nc.tensor.matmul(
                            out=ps[:],
                            lhsT=wt_g[:, blk * D_OUT + h * 128:
                                      blk * D_OUT + (h + 1) * 128],
                            rhs=xt_g[:, blk * B:(blk + 1) * B],
                            start=(blk == 0), stop=(blk == KT - 1))
                    rt = epool.tile([128, B], f32, tag="rt")
                    nc.scalar.activation(
                        out=rt[:], in_=ps[:],
                        func=mybir.ActivationFunctionType.Relu,
                        scale=float(1.0 - ALPHA),
                        bias=cst_t[:, gh:gh + 1])
                    ot = epool.tile([128, B], f32, tag="ot")
                    nc.vector.scalar_tensor_tensor(
                        out=ot[:], in0=ps[:], scalar=ALPHA, in1=rt[:],
                        op0=mybir.AluOpType.mult, op1=mybir.AluOpType.add)
                    nc.vector.tensor_scalar(
                        out=ot[:], in0=ot[:],
                        scalar1=cst_t[:, 24 + h:25 + h],
                        scalar2=cst_t[:, 12 + gh:13 + gh],
                        op0=mybir.AluOpType.mult,
                        op1=mybir.AluOpType.add)
                    nc.sync.dma_start(
                        out=out[gh * 128:(gh + 1) * 128, :], in_=ot[:])
    nc.compile()
    return nc


def _get_program(use_bf16: bool):
    key = (use_bf16, REPEAT)
    if key not in _prog_cache:
        _prog_cache[key] = _build_program(use_bf16)
    return _prog_cache[key]


def _prep_inputs(x, gidx, W, b, gamma, beta, mmean, mvar):
    dtx = _np_dtx()
    xT = np.ascontiguousarray(x.T)  # [D_IN, B]
    assign = [list(range(0, 6))] + \
             [list(range(6 + 5 * i, 6 + 5 * (i + 1))) for i in range(7)]
    inv = (gamma.astype(np.float64)
           / np.sqrt(mvar.astype(np.float64) + BN_EPS))
    caff = beta.astype(np.float64) - mmean.astype(np.float64) * inv
    in_maps, metas = [], []
    for c in range(N_CORES):
        gs = assign[c]
        real = len(gs)
        gs6 = gs + [gs[-1]] * (NG - real)
        flat = gidx[gs6].reshape(-1)          # [NG*G] k-major order
        xgc = xT[flat]                        # [NG*G, B] host gather
        xg_p = np.ascontiguousarray(
            xgc.reshape(NG * KT, 128, B).transpose(1, 0, 2).astype(dtx)
        ).reshape(128, NG * KT * B)
        wc = W[gs6].reshape(NG * G, D_OUT)
        wt_p = np.ascontiguousarray(
            wc.reshape(NG * KT, 128, D_OUT).transpose(1, 0, 2).astype(dtx)
        ).reshape(128, NG * KT * D_OUT)
        # const table [128, 26]: per (g,h) column biasS=(1-a)*b and
        # c2 = c + a*b*inv; plus per-h column inv
        bgh = b[gs6].astype(np.float64).reshape(NG * 2, 128)   # (g,h) x o_r
        invh = inv.reshape(2, 128)
        c2 = caff.reshape(2, 128)[None, :, :] + ALPHA * \
            bgh.reshape(NG, 2, 128) * invh[None, :, :]         # [NG,2,128]
        cst = np.empty((128, NCOL), np.float32)
        cst[:, 0:NG * 2] = ((1.0 - ALPHA) * bgh).T
        cst[:, NG * 2:NG * 4] = c2.reshape(NG * 2, 128).T
        cst[:, NG * 4:NG * 4 + 2] = invh.T
        in_maps.append({"xg": xg_p, "wt": wt_p,
                        "cst": np.ascontiguousarray(cst)})
        metas.append((gs, real))
    return in_maps, metas


def kernel(**inputs):
    x = np.asarray(inputs["x"], dtype=np.float32)
    gidx = np.asarray(inputs["group_idx"]).astype(np.int64)
    W = np.asarray(inputs["W"], dtype=np.float32)
    b = np.asarray(inputs["b"], dtype=np.float32)
    gamma = np.asarray(inputs["gamma"], dtype=np.float32)
    beta = np.asarray(inputs["beta"], dtype=np.float32)
    mmean = np.asarray(inputs["moving_mean"], dtype=np.float32)
    mvar = np.asarray(inputs["moving_var"], dtype=np.float32)

    in_maps, metas = _prep_inputs(x, gidx, W, b, gamma, beta, mmean, mvar)
    nc = _get_program(USE_BF16)

    from concourse import bass_utils
    res = bass_utils.run_bass_kernel_spmd(
        nc, in_maps, core_ids=list(range(N_CORES)), trace=TRACE, **TRACE_KW)
    if TRACE:
        kernel.last_result = res

    full = np.empty((B, N_GROUPS, D_OUT), dtype=np.float32)
    for c, (gs, real) in enumerate(metas):
        o = res.results[c]["out"].reshape(NG, 2, 128, B)
        o = o.transpose(3, 0, 1, 2).reshape(B, NG, D_OUT)
        full[:, gs, :] = o[:, :real, :]
    return full


def run_sim(core=0):
    """CoreSim validation of one core's program (no hardware)."""
    import sys
    sys.path.insert(0, "/root/problem")
    from test import load_ref
    from concourse.bass_interp import CoreSim
    inputs, expected = load_ref()
    x = inputs["x"].astype(np.float32)
    gidx = inputs["group_idx"].astype(np.int64)
    in_maps, metas = _prep_inputs(
        x, gidx, inputs["W"].astype(np.float32), inputs["b"].astype(np.float32),
        inputs["gamma"].astype(np.float32), inputs["beta"].astype(np.float32),
        inputs["moving_mean"].astype(np.float32),
        inputs["moving_var"].astype(np.float32))
    nc = _get_program(USE_BF16)
    sim = CoreSim(nc)
    sim.assign_tensors(in_maps[core])
    sim.simulate(check_with_hw=False)
    o = sim.tensor("out").reshape(NG, 2, 128, B)
    o = o.transpose(3, 0, 1, 2).reshape(B, NG, D_OUT)
    gs, real = metas[core]
    exp_c = expected[:, gs, :]
    act_c = o[:, :real, :]
    err = np.max(np.abs(act_c - exp_c)) / (np.max(np.abs(exp_c)) + 1e-30)
    print(f"core {core}: sim max-abs-rel err = {err:.3e}")
    return err


if __name__ == "__main__":
    run_sim(0)


# revision 8
# speedup vs baseline: 1.0954x; 1.0954x over previous
"""Trainium2 Bass kernel for nn_LocallyDense (grouped gather + per-group Dense
+ LeakyReLU + BatchNorm inference).

Sharding: expert-parallel over the 41 groups across 8 cores (6 groups/core,
padded with a duplicate group on 5-group cores so one SPMD program fits all).

The gather runs on the HOST during sharding prep: each core receives its
groups' x-columns already gathered AND packed into tile layout
([128, NG*KT*B]: partition = k%128, free = (k-block, batch)), so the device
program is a pure streaming grouped GEMM with a TRANSPOSED output
(psum[o, b] via lhsT=W): per-output-channel constants (bias, BN scale/shift)
are then per-partition scalars riding the ACT/DVE instructions — no
broadcasts, no bias matmuls.

Per output half (z = x@W + b, p = psum = x@W):
  rt = Relu((1-a)*p + (1-a)*b)        # ACT, per-partition bias AP
  ot = a*p + rt = leaky(z) - a*b      # DVE scalar_tensor_tensor
  y  = ot*inv + (c + a*b*inv)         # DVE tensor_scalar, per-partition APs
where inv = gamma/sqrt(var+eps), c = beta - mean*inv (host-computed).
"""

import numpy as np
import ml_dtypes

B, D_IN, N_GROUPS, G, D_OUT = 256, 65536, 41, 1536, 256
BN_EPS = 1e-3
ALPHA = 0.3
N_CORES = 8
NG = 6                # groups per core (padded)
KT = G // 128         # 12 K-tiles per group
NCOL = NG * 2 * 2 + 2  # const table columns: biasS | c2 | inv

USE_BF16 = True       # x/W feed the PE in bf16 (fp32 accumulate in PSUM)
TRACE = False         # set by test.py for profiling runs
TRACE_KW = {}
REPEAT = 1

_prog_cache = {}


def _np_dtx():
    return ml_dtypes.bfloat16 if USE_BF16 else np.float32


def _build_program(use_bf16: bool):
    import concourse.bacc as bacc
    import concourse.mybir as mybir
    import concourse.tile as tile

    f32 = mybir.dt.float32
    dt_x = mybir.dt.bfloat16 if use_bf16 else mybir.dt.float32

    nc = bacc.Bacc("TRN2", target_bir_lowering=False, debug=False,
                   num_devices=N_CORES)
    xg = nc.dram_tensor("xg", [128, NG * KT * B], dt_x, kind="ExternalInput")
    wt = nc.dram_tensor("wt", [128, NG * KT * D_OUT], dt_x,
                        kind="ExternalInput")
    # columns: [0:12] biasS=(1-a)*b, [12:24] c2=c+a*b*inv, [24:26] inv
    cst = nc.dram_tensor("cst", [128, NCOL], f32, kind="ExternalInput")
    out = nc.dram_tensor("out", [NG * 2 * 128, B], f32, kind="ExternalOutput")

    with tile.TileContext(nc) as tc:
        with tc.tile_pool(name="const", bufs=1) as cpool, \
             tc.tile_pool(name="xp", bufs=3) as xpool, \
             tc.tile_pool(name="wp", bufs=3) as wpool, \
             tc.tile_pool(name="ep", bufs=4) as epool, \
             tc.tile_pool(name="ps", bufs=3, space="PSUM") as ppool:

            cst_t = cpool.tile([128, NCOL], f32)
            nc.sync.dma_start(out=cst_t[:], in_=cst[:, :])

            for g_rep in range(REPEAT * NG):
                g = g_rep % NG
                xt_g = xpool.tile([128, KT * B], dt_x, tag="xg")
                nc.sync.dma_start(
                    out=xt_g[:], in_=xg[:, g * KT * B:(g + 1) * KT * B])
                wt_g = wpool.tile([128, KT * D_OUT], dt_x, tag="wg")
                nc.scalar.dma_start(
                    out=wt_g[:], in_=wt[:, g * KT * D_OUT:(g + 1) * KT * D_OUT])
                for h in range(2):
                    gh = g * 2 + h
                    ps = ppool.tile([128, B], f32, tag=f"ps{h}",
                                    name=f"ps{h}_{g_rep}")
                    for blk in range(KT):
                        nc.tensor.matmul(
                            out=ps[:],
                            lhsT=wt_g[:, blk * D_OUT + h * 128:
                                      blk * D_OUT + (h + 1) * 128],
                            rhs=xt_g[:, blk * B:(blk + 1) * B],
                            start=(blk == 0), stop=(blk == KT - 1))
                    rt = epool.tile([128, B], f32, tag="rt")
                    nc.scalar.activation(
                        out=rt[:], in_=ps[:],
                        func=mybir.ActivationFunctionType.Relu,
                        scale=float(1.0 - ALPHA),
                        bias=cst_t[:, gh:gh + 1])
                    ot = epool.tile([128, B], f32, tag="ot")
                    nc.vector.scalar_tensor_tensor(
                        out=ot[:], in0=ps[:], scalar=ALPHA, in1=rt[:],
                        op0=mybir.AluOpType.mult, op1=mybir.AluOpType.add)
                    nc.vector.tensor_scalar(
                        out=ot[:], in0=ot[:],
                        scalar1=cst_t[:, 24 + h:25 + h],
                        scalar2=cst_t[:, 12 + gh:13 + gh],
                        op0=mybir.AluOpType.mult,
                        op1=mybir.AluOpType.add)
                    nc.gpsimd.dma_start(
                        out=out[gh * 128:(gh + 1) * 128, :], in_=ot[:])
    nc.compile()
    return nc


def _get_program(use_bf16: bool):
    key = (use_bf16, REPEAT)
    if key not in _prog_cache:
        _prog_cache[key] = _build_program(use_bf16)
    return _prog_cache[key]


def _prep_inputs(x, gidx, W, b, gamma, beta, mmean, mvar):
    dtx = _np_dtx()
    xT = np.ascontiguousarray(x.T)  # [D_IN, B]
    assign = [list(range(0, 6))] + \
             [list(range(6 + 5 * i, 6 + 5 * (i + 1))) for i in range(7)]
    inv = (gamma.astype(np.float64)
           / np.sqrt(mvar.astype(np.float64) + BN_EPS))
    caff = beta.astype(np.float64) - mmean.astype(np.float64) * inv
    in_maps, metas = [], []
    for c in range(N_CORES):
        gs = assign[c]
        real = len(gs)
        gs6 = gs + [gs[-1]] * (NG - real)
        flat = gidx[gs6].reshape(-1)          # [NG*G] k-major order
        xgc = xT[flat]                        # [NG*G, B] host gather
        xg_p = np.ascontiguousarray(
            xgc.reshape(NG * KT, 128, B).transpose(1, 0, 2).astype(dtx)
        ).reshape(128, NG * KT * B)
        wc = W[gs6].reshape(NG * G, D_OUT)
        wt_p = np.ascontiguousarray(
            wc.reshape(NG * KT, 128, D_OUT).transpose(1, 0, 2).astype(dtx)
        ).reshape(128, NG * KT * D_OUT)
        # const table [128, 26]: per (g,h) column biasS=(1-a)*b and
        # c2 = c + a*b*inv; plus per-h column inv
        bgh = b[gs6].astype(np.float64).reshape(NG * 2, 128)   # (g,h) x o_r
        invh = inv.reshape(2, 128)
        c2 = caff.reshape(2, 128)[None, :, :] + ALPHA * \
            bgh.reshape(NG, 2, 128) * invh[None, :, :]         # [NG,2,128]
        cst = np.empty((128, NCOL), np.float32)
        cst[:, 0:NG * 2] = ((1.0 - ALPHA) * bgh).T
        cst[:, NG * 2:NG * 4] = c2.reshape(NG * 2, 128).T
        cst[:, NG * 4:NG * 4 + 2] = invh.T
        in_maps.append({"xg": xg_p, "wt": wt_p,
                        "cst": np.ascontiguousarray(cst)})
        metas.append((gs, real))
    return in_maps, metas


def kernel(**inputs):
    x = np.asarray(inputs["x"], dtype=np.float32)
    gidx = np.asarray(inputs["group_idx"]).astype(np.int64)
    W = np.asarray(inputs["W"], dtype=np.float32)
    b = np.asarray(inputs["b"], dtype=np.float32)
    gamma = np.asarray(inputs["gamma"], dtype=np.float32)
    beta = np.asarray(inputs["beta"], dtype=np.float32)
    mmean = np.asarray(inputs["moving_mean"], dtype=np.float32)
    mvar = np.asarray(inputs["moving_var"], dtype=np.float32)

    in_maps, metas = _prep_inputs(x, gidx, W, b, gamma, beta, mmean, mvar)
    nc = _get_program(USE_BF16)

    from concourse import bass_utils
    res = bass_utils.run_bass_kernel_spmd(
        nc, in_maps, core_ids=list(range(N_CORES)), trace=TRACE, **TRACE_KW)
    if TRACE:
        kernel.last_result = res

    full = np.empty((B, N_GROUPS, D_OUT), dtype=np.float32)
    for c, (gs, real) in enumerate(metas):
        o = res.results[c]["out"].reshape(NG, 2, 128, B)
        o = o.transpose(3, 0, 1, 2).reshape(B, NG, D_OUT)
        full[:, gs, :] = o[:, :real, :]
    return full


def run_sim(core=0):
    """CoreSim validation of one core's program (no hardware)."""
    import sys
    sys.path.insert(0, "/root/problem")
    from test import load_ref
    from concourse.bass_interp import CoreSim
    inputs, expected = load_ref()
    x = inputs["x"].astype(np.float32)
    gidx = inputs["group_idx"].astype(np.int64)
    in_maps, metas = _prep_inputs(
        x, gidx, inputs["W"].astype(np.float32), inputs["b"].astype(np.float32),
        inputs["gamma"].astype(np.float32), inputs["beta"].astype(np.float32),
        inputs["moving_mean"].astype(np.float32),
        inputs["moving_var"].astype(np.float32))
    nc = _get_program(USE_BF16)
    sim = CoreSim(nc)
    sim.assign_tensors(in_maps[core])
    sim.simulate(check_with_hw=False)
    o = sim.tensor("out").reshape(NG, 2, 128, B)
    o = o.transpose(3, 0, 1, 2).reshape(B, NG, D_OUT)
    gs, real = metas[core]
    exp_c = expected[:, gs, :]
    act_c = o[:, :real, :]
    err = np.max(np.abs(act_c - exp_c)) / (np.max(np.abs(exp_c)) + 1e-30)
    print(f"core {core}: sim max-abs-rel err = {err:.3e}")
    return err


if __name__ == "__main__":
    run_sim(0)
